# revision 16
# baseline (speedup 1.0000x reference)
"""TRN2 Bass kernel for nn_LiveNet: y = relu(relu(x @ W1.T + b1) @ W2.T + b2).

Full shapes: x [65536, 1024] f32, W1 [256, 1024], b1 [256], W2 [64, 256], b2 [64].
Sharding: pure data parallel over batch across 8 NeuronCores (8192 rows each);
weights replicated; no cross-device communication.

Layout strategy: the PE contracts along the partition dim, so the x-side
operand must be feature-major. The host hands each core its batch shard
pre-transposed into a partition-major slab layout (one contiguous 16 KiB
DRAM run per (partition, slab) -> 128 DMA descriptors per slab load),
making the device side a pure stream: DMA x slabs -> L1 matmuls (f32r,
1 cyc/row) accumulating h.T in PSUM -> ACT relu+bias -> L2 matmuls ->
ACT relu+bias -> DMA y.T out. Matmuls run in float32r (fp32 with 11-bit
mantissa, ~1.2e-4 unit roundoff, 4x the throughput of fp32 on the PE);
the f32->f32r rounding happens in-flight in the gpsimd (SWDGE) DMA.

Measured on TRN2 (8 cores, repeats-delta timing): ~101-105 us/pass vs a
~100 us contended-HBM roofline (33.5 MB x read + 2 MB y write per core at
~358 GB/s/core). Relative error vs fp32 reference: ~1.2e-4 of absmax.
"""
import numpy as np

N_INPUTS = 1024
N_MIDDLE = 256
N_OUTPUTS = 64
BATCH = 65536
N_CORES = 8
B_LOC = BATCH // N_CORES          # 8192
G = 512                           # batch-group (one PSUM bank of fp32)
NG = B_LOC // G                   # 16 groups
NK1 = N_INPUTS // 128             # 8 k-chunks layer 1
NM = N_MIDDLE // 128              # 2 m-tiles
NK2 = N_MIDDLE // 128             # 2 k-chunks layer 2

_COMPILED = None


def _build(mode="swdge", repeats=1, groups_per_load=1, xtr_bufs=4,
           ph_bufs=6, split_loads=1):
    """Build the per-core Bass program.

    mode:
      "swdge": x slabs loaded by gpsimd DMA casting f32->f32r in flight.
      "dve":   x slabs loaded f32 by HWDGE; DVE copy casts to f32r.
    repeats: run the whole pass N times (benchmarking only).
    groups_per_load: batch-groups (of 512) fetched per x DMA.
    """
    import concourse.bacc as bacc
    import concourse.tile as tile
    import concourse.mybir as mybir

    F32 = mybir.dt.float32
    F32R = mybir.dt.float32r
    RELU = mybir.ActivationFunctionType.Relu

    GL = groups_per_load
    BL = G * GL                     # batch columns per load
    assert NG % GL == 0

    nc = bacc.Bacc("TRN2", target_bir_lowering=False, debug=False,
                   enable_asserts=True, num_devices=N_CORES)

    # xh[p, s, k, b] = x_core[s*BL + b, k*128 + p]: partition-major so each
    # (partition, slab) is one contiguous 32 KiB DRAM run -> 128 DMA
    # descriptors per slab load.
    NSLAB = NG // GL
    xt_d = nc.dram_tensor("xh", (128, NSLAB * NK1 * BL), F32,
                          kind="ExternalInput")
    w1t_d = nc.dram_tensor("w1t", (N_INPUTS, N_MIDDLE), F32, kind="ExternalInput")
    w2t_d = nc.dram_tensor("w2t", (N_MIDDLE, N_OUTPUTS), F32, kind="ExternalInput")
    b1_d = nc.dram_tensor("b1s", (128, NM), F32, kind="ExternalInput")
    b2_d = nc.dram_tensor("b2s", (N_OUTPUTS, 1), F32, kind="ExternalInput")
    yt_d = nc.dram_tensor("yt", (N_OUTPUTS, B_LOC), F32, kind="ExternalOutput")

    with tile.TileContext(nc) as tc:
        with (
            tc.tile_pool(name="const", bufs=1) as cpool,
            tc.tile_pool(name="xg", bufs=2) as xg_pool,
            tc.tile_pool(name="xtr", bufs=xtr_bufs) as xtr_pool,
            tc.tile_pool(name="h", bufs=4) as h_pool,
            tc.tile_pool(name="y", bufs=3) as y_pool,
            tc.tile_pool(name="ph", bufs=ph_bufs, space="PSUM") as ph_pool,
            tc.tile_pool(name="py", bufs=2, space="PSUM") as py_pool,
        ):
            # ---- constants (loaded once) ----
            # Weights loaded f32 via HWDGE (keeps the gpsimd queue free for
            # the first x slab), rounded to f32r by one DVE copy each.
            w1f = cpool.tile([128, NK1 * N_MIDDLE], F32, tag="w1f")
            w2f = cpool.tile([128, NK2 * N_OUTPUTS], F32, tag="w2f")
            w1r = cpool.tile([128, NK1 * N_MIDDLE], F32R, tag="w1r")
            w2r = cpool.tile([128, NK2 * N_OUTPUTS], F32R, tag="w2r")
            b1_sb = cpool.tile([128, NM], F32, tag="b1")
            b2_sb = cpool.tile([N_OUTPUTS, 1], F32, tag="b2")

            nc.sync.dma_start(
                w1f[:].rearrange("p (k m) -> p k m", k=NK1),
                w1t_d.ap().rearrange("(k p) m -> p k m", p=128))
            nc.sync.dma_start(
                w2f[:].rearrange("p (k o) -> p k o", k=NK2),
                w2t_d.ap().rearrange("(k p) o -> p k o", p=128))
            nc.vector.tensor_copy(w1r[:], w1f[:])
            nc.vector.tensor_copy(w2r[:], w2f[:])
            nc.sync.dma_start(b1_sb[:], b1_d.ap())
            nc.sync.dma_start(b2_sb[:], b2_d.ap())

            for _rep in range(repeats):
              for lg in range(NSLAB):
                # ---- load x slab [128, NK1*BL]: 1 contiguous run/partition ----
                xtr_t = xtr_pool.tile([128, NK1 * BL], F32R, tag="xtr")
                SL = NK1 * BL // split_loads
                for sp in range(split_loads):
                    src = xt_d.ap()[:, lg * (NK1 * BL) + sp * SL:
                                    lg * (NK1 * BL) + (sp + 1) * SL]
                    if mode == "swdge":
                        # gpsimd DMA casts f32 -> f32r in flight
                        nc.gpsimd.dma_start(xtr_t[:, sp * SL:(sp + 1) * SL], src)
                    else:
                        xg_t = xg_pool.tile([128, NK1 * BL], F32, tag="xg")
                        nc.sync.dma_start(xg_t[:, sp * SL:(sp + 1) * SL], src)
                        nc.vector.tensor_copy(
                            xtr_t[:, sp * SL:(sp + 1) * SL],
                            xg_t[:, sp * SL:(sp + 1) * SL])

                for sub in range(GL):
                    g = lg * GL + sub
                    # ---- layer 1: h.T = relu(W1 @ x.T + b1) ----
                    h_ts = []
                    for mc in range(NM):
                        ph = ph_pool.tile([128, G], F32, tag="ph")
                        for k in range(NK1):
                            nc.tensor.matmul(
                                ph[:],
                                w1r[:, k * N_MIDDLE + mc * 128:
                                    k * N_MIDDLE + (mc + 1) * 128],
                                xtr_t[:, k * BL + sub * G:
                                      k * BL + (sub + 1) * G],
                                start=(k == 0), stop=(k == NK1 - 1))
                        h_t = h_pool.tile([128, G], F32R, tag="h")
                        nc.scalar.activation(h_t[:], ph[:], RELU,
                                             bias=b1_sb[:, mc:mc + 1])
                        h_ts.append(h_t)

                    # ---- layer 2: y.T = relu(W2 @ h.T + b2) ----
                    py = py_pool.tile([N_OUTPUTS, G], F32, tag="py")
                    for kc in range(NK2):
                        nc.tensor.matmul(
                            py[:],
                            w2r[:, kc * N_OUTPUTS:(kc + 1) * N_OUTPUTS],
                            h_ts[kc][:],
                            start=(kc == 0), stop=(kc == NK2 - 1))
                    y_t = y_pool.tile([N_OUTPUTS, G], F32, tag="y")
                    nc.scalar.activation(y_t[:], py[:], RELU,
                                         bias=b2_sb[:, 0:1])
                    nc.sync.dma_start(yt_d.ap()[:, g * G:(g + 1) * G], y_t[:])

    nc.compile()
    return nc


def _get_compiled():
    global _COMPILED
    if _COMPILED is None:
        _COMPILED = _build()
    return _COMPILED


def make_in_maps(inputs, groups_per_load=1):
    x = np.asarray(inputs["x"], dtype=np.float32)
    W1 = np.asarray(inputs["W1"], dtype=np.float32)
    W2 = np.asarray(inputs["W2"], dtype=np.float32)
    b1 = np.asarray(inputs["b1"], dtype=np.float32)
    b2 = np.asarray(inputs["b2"], dtype=np.float32)

    # per-core shards, partition-major slab layout (host-side layout step):
    # xh[c, p, s, k, b] = x[c*B_LOC + s*BL + b, k*128 + p]
    GL = groups_per_load
    BL = G * GL
    NSLAB = NG // GL
    xh = np.ascontiguousarray(
        x.reshape(N_CORES, NSLAB, BL, NK1, 128).transpose(0, 4, 1, 3, 2)
    ).reshape(N_CORES, 128, NSLAB * NK1 * BL)
    w1t = np.ascontiguousarray(W1.T)                      # [1024, 256]
    w2t = np.ascontiguousarray(W2.T)                      # [256, 64]
    b1s = np.ascontiguousarray(b1.reshape(NM, 128).T)     # [128, 2]
    b2s = np.ascontiguousarray(b2.reshape(N_OUTPUTS, 1))  # [64, 1]
    return [
        {"xh": xh[i], "w1t": w1t, "w2t": w2t, "b1s": b1s, "b2s": b2s}
        for i in range(N_CORES)
    ]


def run_full(inputs, trace=False):
    """Run on 8 cores. Returns (y [65536, 64] f32, BassKernelResults)."""
    from concourse.bass_utils import run_bass_kernel_spmd

    nc = _get_compiled()
    in_maps = make_in_maps(inputs)
    try:
        res = run_bass_kernel_spmd(nc, in_maps, core_ids=list(range(N_CORES)),
                                   trace=trace)
    except ModuleNotFoundError:
        # axon NTFF profiling hook unavailable in this environment
        res = run_bass_kernel_spmd(nc, in_maps, core_ids=list(range(N_CORES)),
                                   trace=False)
    y = np.concatenate(
        [res.results[i]["yt"].T for i in range(N_CORES)], axis=0)
    return np.ascontiguousarray(y), res


def kernel(**inputs) -> np.ndarray:
    return run_full(inputs)[0]
